# revision 7
# baseline (speedup 1.0000x reference)
"""Multi-head attention (B=8, N=2048, D=512, H=8, dh=64) on 8 TRN2 NeuronCores.

Strategy: pure data parallelism — one batch element per core. Per core:
  xT = x.T                       (PE transposes, 128x128 blocks)
  qT = (Wq*sel/8).T @ xT         kT = (Wk*sel).T @ xT        [512, 2048]
  v  = x @ (Wv*sel)              [2048, 512] natural layout
  per (head, n-half, m-tile):
     dotsT = k_h @ q_h.T tile    [128m, 1024n]  (PSUM, f32r matmuls)
     attnT = exp(dotsT)          (ScalarE, no max-subtraction: |dots|<~1.5)
     po[0:64]   += v_h.T @ attnT      (out_hT, unnormalized)
     po[64:128] += ones.T @ attnT     (col-tiled concurrent matmul -> 64
                                       replicated rows of softmax sums)
  outT_h = po[0:64] / po[64:128]  (aligned DVE divide, no broadcast)
  y = outT.T @ Wo + bo            -> out [2048, 512]

sel, the 1/sqrt(dh) scale, and bias handling are folded host-side into the
weights. All matmul operands are bitcast to float32r (full fp32 storage,
full-rate PE streaming).
"""

import numpy as np

B, N, DIM = 8, 2048, 512
HEADS, DHEAD = 8, 64
P = 128
NT = N // P      # 16 row tiles
KC = DIM // P    # 4 contraction chunks
NCORES = 8

_CACHE = {}


def _build():
    from contextlib import ExitStack

    import concourse.bass as bass
    import concourse.mybir as mybir
    import concourse.tile as tile
    from concourse import bacc
    from concourse.masks import make_identity

    F32 = mybir.dt.float32
    F32R = mybir.dt.float32r
    EXP = mybir.ActivationFunctionType.Exp
    MULT = mybir.AluOpType.mult
    ADD = mybir.AluOpType.add
    BF16 = mybir.dt.bfloat16

    def r(ap):
        return ap.bitcast(F32R)

    nc = bacc.Bacc("TRN2", target_bir_lowering=False, debug=False,
                   num_devices=NCORES)

    x_d = nc.declare_dram_parameter("x", [N, DIM], F32, isOutput=False)
    wq_d = nc.declare_dram_parameter("Wq", [DIM, DIM], F32, isOutput=False)
    wk_d = nc.declare_dram_parameter("Wk", [DIM, DIM], F32, isOutput=False)
    wv_d = nc.declare_dram_parameter("Wv", [DIM, DIM], F32, isOutput=False)
    wo_d = nc.declare_dram_parameter("Wo", [DIM, DIM], F32, isOutput=False)
    bo_d = nc.declare_dram_parameter("bo", [1, DIM], F32, isOutput=False)
    out_d = nc.declare_dram_parameter("out", [N, DIM], F32, isOutput=True)

    with ExitStack() as ctx:
        tc = ctx.enter_context(tile.TileContext(nc))
        const = ctx.enter_context(tc.tile_pool(name="const", bufs=1))
        persist = ctx.enter_context(tc.tile_pool(name="persist", bufs=1))
        xin = ctx.enter_context(tc.tile_pool(name="xin", bufs=3))
        attnp = ctx.enter_context(tc.tile_pool(name="attnp", bufs=2))
        yout = ctx.enter_context(tc.tile_pool(name="yout", bufs=3))
        # PSUM: mm (phases 1/2/4) 2 banks + dots 2x2 banks + po 2 banks = 8
        mmp = ctx.enter_context(tc.tile_pool(name="mmp", bufs=2, space="PSUM"))
        dotsp = ctx.enter_context(tc.tile_pool(name="dotsp", bufs=2, space="PSUM"))
        pop = ctx.enter_context(tc.tile_pool(name="pop", bufs=2, space="PSUM"))

        ident = const.tile([P, P], F32)
        make_identity(nc, ident)
        ones_f32 = const.tile([P, 64], F32)
        nc.vector.memset(ones_f32, 1.0)
        ones64 = const.tile([P, 64], BF16)
        nc.vector.tensor_copy(ones64[:], ones_f32[:])
        bo_bc = const.tile([P, DIM], F32)
        bo_ap = bo_d.ap()
        nc.gpsimd.dma_start(
            out=bo_bc,
            in_=bass.AP(tensor=bo_ap.tensor, offset=bo_ap.offset,
                        ap=[[0, P], [1, DIM]]),
        )

        # Persistent SBUF arrays (tags give each logical array its own slots)
        xT = [persist.tile([P, N], F32R, tag="xot", bufs=4, name=f"xT{i}") for i in range(KC)]
        qT = [persist.tile([P, N], F32R, tag="qT", bufs=4, name=f"qT{i}") for i in range(KC)]
        kT = [persist.tile([P, N], F32R, tag="kT", bufs=4, name=f"kT{i}") for i in range(KC)]
        v_sb = [persist.tile([P, DIM], BF16, tag="v", bufs=NT, name=f"v{i}") for i in range(NT)]
        w_sb = {}
        for wname, wd in (("q", wq_d), ("k", wk_d), ("v", wv_d), ("o", wo_d)):
            tiles = []
            for c in range(KC):
                stage = xin.tile([P, DIM], F32, tag="wstage", bufs=2,
                                 name=f"ws{wname}{c}")
                nc.sync.dma_start(stage[:], wd[c * P:(c + 1) * P, :])
                t = persist.tile([P, DIM], F32R, tag="w", bufs=16, name=f"w{wname}{c}")
                nc.vector.tensor_copy(t[:], stage[:])
                tiles.append(t)
            w_sb[wname] = tiles

        # ---- Phase 1: load x and transpose into xT ----
        for mt in range(NT):
            xt_in = xin.tile([P, DIM], F32)
            nc.sync.dma_start(xt_in[:], x_d[mt * P:(mt + 1) * P, :])
            for c in range(KC):
                tp = mmp.tile([P, P], F32, tag="mm")
                nc.tensor.transpose(tp[:], xt_in[:, c * P:(c + 1) * P], ident[:])
                nc.vector.tensor_copy(xT[c][:, mt * P:(mt + 1) * P], tp[:])

        # ---- Phase 2: projections ----
        # v = x @ Wv'  (natural layout, per m-tile)
        for mt in range(NT):
            pv = mmp.tile([P, DIM], F32, tag="mm")
            for c in range(KC):
                nc.tensor.matmul(pv[:], (xT[c][:, mt * P:(mt + 1) * P]),
                                 (w_sb["v"][c][:]),
                                 start=(c == 0), stop=(c == KC - 1))
            nc.vector.tensor_copy(v_sb[mt][:], pv[:])
        # qT = Wq'.T @ xT ; kT = Wk'.T @ xT   (per inner-chunk p, n-chunk)
        for wname, dst in (("q", qT), ("k", kT)):
            for p in range(KC):
                for nck in range(KC):
                    pq = mmp.tile([P, DIM], F32, tag="mm")
                    for c in range(KC):
                        nc.tensor.matmul(
                            pq[:],
                            (w_sb[wname][c][:, p * P:(p + 1) * P]),
                            (xT[c][:, nck * DIM:(nck + 1) * DIM]),
                            start=(c == 0), stop=(c == KC - 1))
                    nc.vector.tensor_copy(dst[p][:, nck * DIM:(nck + 1) * DIM],
                                          pq[:])

        # outT shares slots with xT (xT dead after phase 2)
        outT = [persist.tile([P, N], F32R, tag="xot", bufs=4, name=f"outT{i}") for i in range(KC)]

        # ---- Phase 3: attention ----
        for h in range(HEADS):
            hp, hr = h // 2, (h % 2) * 64
            vh = v_sb  # head h columns: [:, h*64:(h+1)*64]
            for half in range(2):
                off = half * 1024
                po = [pop.tile([P, DIM], F32, tag="po", name=f"po_h{h}_f{half}_{j}") for j in range(2)]
                for mt in range(NT):
                    dts = dotsp.tile([P, 1024], F32, tag="dots")
                    for j in range(2):
                        nc.tensor.matmul(
                            dts[:, j * DIM:(j + 1) * DIM],
                            (kT[hp][hr:hr + 64, mt * P:(mt + 1) * P]),
                            (qT[hp][hr:hr + 64, off + j * DIM:off + (j + 1) * DIM]),
                            start=True, stop=True)
                    at = attnp.tile([P, 1024], BF16, tag="at")
                    nc.scalar.activation(at[:], dts[:], EXP)
                    for j in range(2):
                        nc.tensor.matmul(
                            po[j][0:64, :],
                            (vh[mt][:, h * 64:(h + 1) * 64]),
                            (at[:, j * DIM:(j + 1) * DIM]),
                            start=(mt == 0), stop=(mt == NT - 1),
                            tile_position=(0, 0))
                        nc.tensor.matmul(
                            po[j][64:128, :],
                            (ones64[:]),
                            (at[:, j * DIM:(j + 1) * DIM]),
                            start=(mt == 0), stop=(mt == NT - 1),
                            tile_position=(0, 64))
                for j in range(2):
                    rc = yout.tile([64, DIM], F32, tag="sums", bufs=2,
                                   name=f"rc_h{h}_f{half}_{j}")
                    nc.vector.reciprocal(rc[:], po[j][64:128, :])
                    nc.vector.tensor_tensor(
                        out=outT[hp][hr:hr + 64, off + j * DIM:off + (j + 1) * DIM],
                        in0=po[j][0:64, :], in1=rc[:], op=MULT)

        # ---- Phase 4: y = outT.T @ Wo + bo ----
        for nt in range(NT):
            py = mmp.tile([P, DIM], F32, tag="mm")
            for c in range(KC):
                nc.tensor.matmul(py[:], (outT[c][:, nt * P:(nt + 1) * P]),
                                 (w_sb["o"][c][:]),
                                 start=(c == 0), stop=(c == KC - 1))
            y_sb = yout.tile([P, DIM], F32)
            nc.vector.tensor_tensor(out=y_sb[:], in0=py[:], in1=bo_bc[:], op=ADD)
            nc.sync.dma_start(out_d[nt * P:(nt + 1) * P, :], y_sb[:])

    nc.compile()
    return nc


def _get_nc():
    if "nc" not in _CACHE:
        _CACHE["nc"] = _build()
    return _CACHE["nc"]


def kernel(x, Wq, Wk, Wv, sel, Wo, bo):
    from concourse.bass_utils import run_bass_kernel_spmd

    x = np.asarray(x, dtype=np.float32)
    sel = np.asarray(sel, dtype=np.float32)
    scale = float(DHEAD) ** -0.5
    wq_f = np.ascontiguousarray(np.asarray(Wq, np.float32) * sel[None, :] * scale)
    wk_f = np.ascontiguousarray(np.asarray(Wk, np.float32) * sel[None, :])
    wv_f = np.ascontiguousarray(np.asarray(Wv, np.float32) * sel[None, :])
    wo_f = np.ascontiguousarray(np.asarray(Wo, np.float32))
    bo_f = np.ascontiguousarray(np.asarray(bo, np.float32).reshape(1, DIM))

    nc = _get_nc()
    in_maps = [
        {"x": np.ascontiguousarray(x[b]), "Wq": wq_f, "Wk": wk_f,
         "Wv": wv_f, "Wo": wo_f, "bo": bo_f}
        for b in range(B)
    ]
    res = run_bass_kernel_spmd(nc, in_maps, core_ids=list(range(NCORES)))
    return np.stack([res.results[b]["out"] for b in range(B)], axis=0)


# revision 8
# speedup vs baseline: 1.3053x; 1.3053x over previous
"""Multi-head attention (B=8, N=2048, D=512, H=8, dh=64) on 8 TRN2 NeuronCores.

Strategy: pure data parallelism — one batch element per core. Per core:
  xT = x.T                       (PE transposes, 128x128 blocks)
  qT = (Wq*sel/8).T @ xT         kT = (Wk*sel).T @ xT        [512, 2048]
  v  = x @ (Wv*sel)              [2048, 512] natural layout
  per (head, n-half, m-tile):
     dotsT = k_h @ q_h.T tile    [128m, 1024n]  (PSUM, f32r matmuls)
     attnT = exp(dotsT)          (ScalarE, no max-subtraction: |dots|<~1.5)
     po[0:64]   += v_h.T @ attnT      (out_hT, unnormalized)
     po[64:128] += ones.T @ attnT     (col-tiled concurrent matmul -> 64
                                       replicated rows of softmax sums)
  outT_h = po[0:64] / po[64:128]  (aligned DVE divide, no broadcast)
  y = outT.T @ Wo + bo            -> out [2048, 512]

sel, the 1/sqrt(dh) scale, and bias handling are folded host-side into the
weights. All matmul operands are bitcast to float32r (full fp32 storage,
full-rate PE streaming).
"""

import numpy as np

B, N, DIM = 8, 2048, 512
HEADS, DHEAD = 8, 64
P = 128
NT = N // P      # 16 row tiles
KC = DIM // P    # 4 contraction chunks
NCORES = 8

_CACHE = {}


def _build():
    from contextlib import ExitStack

    import concourse.bass as bass
    import concourse.mybir as mybir
    import concourse.tile as tile
    from concourse import bacc
    from concourse.masks import make_identity

    F32 = mybir.dt.float32
    F32R = mybir.dt.float32r
    EXP = mybir.ActivationFunctionType.Exp
    MULT = mybir.AluOpType.mult
    ADD = mybir.AluOpType.add
    BF16 = mybir.dt.bfloat16

    def r(ap):
        return ap.bitcast(F32R)

    nc = bacc.Bacc("TRN2", target_bir_lowering=False, debug=False,
                   num_devices=NCORES)

    x_d = nc.declare_dram_parameter("x", [N, DIM], F32, isOutput=False)
    wq_d = nc.declare_dram_parameter("Wq", [DIM, DIM], F32, isOutput=False)
    wk_d = nc.declare_dram_parameter("Wk", [DIM, DIM], F32, isOutput=False)
    wv_d = nc.declare_dram_parameter("Wv", [DIM, DIM], F32, isOutput=False)
    wo_d = nc.declare_dram_parameter("Wo", [DIM, DIM], F32, isOutput=False)
    bo_d = nc.declare_dram_parameter("bo", [1, DIM], F32, isOutput=False)
    out_d = nc.declare_dram_parameter("out", [N, DIM], F32, isOutput=True)

    with ExitStack() as ctx:
        tc = ctx.enter_context(tile.TileContext(nc))
        const = ctx.enter_context(tc.tile_pool(name="const", bufs=1))
        persist = ctx.enter_context(tc.tile_pool(name="persist", bufs=1))
        xin = ctx.enter_context(tc.tile_pool(name="xin", bufs=3))
        attnp = ctx.enter_context(tc.tile_pool(name="attnp", bufs=3))
        yout = ctx.enter_context(tc.tile_pool(name="yout", bufs=3))
        # PSUM: "dots" tag 2x[128,1024] slots (also reused for proj/y
        # [128,512] psums) = 4 banks + "po" tag 4 banks = 8 banks total.
        psp = ctx.enter_context(tc.tile_pool(name="psp", bufs=2, space="PSUM"))

        ident = const.tile([P, P], F32)
        make_identity(nc, ident)
        ones_f32 = const.tile([P, 64], F32)
        nc.vector.memset(ones_f32, 1.0)
        ones64 = const.tile([P, 64], BF16)
        nc.vector.tensor_copy(ones64[:], ones_f32[:])
        bo_bc = const.tile([P, DIM], F32)
        bo_ap = bo_d.ap()
        nc.gpsimd.dma_start(
            out=bo_bc,
            in_=bass.AP(tensor=bo_ap.tensor, offset=bo_ap.offset,
                        ap=[[0, P], [1, DIM]]),
        )

        # Persistent SBUF arrays (tags give each logical array its own slots)
        xT = [persist.tile([P, N], F32R, tag="xot", bufs=4, name=f"xT{i}") for i in range(KC)]
        qT = [persist.tile([P, N], F32R, tag="qT", bufs=4, name=f"qT{i}") for i in range(KC)]
        kT = [persist.tile([P, N], F32R, tag="kT", bufs=4, name=f"kT{i}") for i in range(KC)]
        v_sb = [persist.tile([P, DIM], BF16, tag="v", bufs=NT, name=f"v{i}") for i in range(NT)]
        w_sb = {}
        for wname, wd in (("q", wq_d), ("k", wk_d), ("v", wv_d), ("o", wo_d)):
            tiles = []
            for c in range(KC):
                stage = xin.tile([P, DIM], F32, tag="wstage", bufs=2,
                                 name=f"ws{wname}{c}")
                nc.sync.dma_start(stage[:], wd[c * P:(c + 1) * P, :])
                t = persist.tile([P, DIM], F32R, tag="w", bufs=16, name=f"w{wname}{c}")
                nc.vector.tensor_copy(t[:], stage[:])
                tiles.append(t)
            w_sb[wname] = tiles

        # ---- Phase 1: load x and transpose into xT ----
        for mt in range(NT):
            xt_in = xin.tile([P, DIM], F32)
            nc.sync.dma_start(xt_in[:], x_d[mt * P:(mt + 1) * P, :])
            for c in range(KC):
                tp = psp.tile([P, P], F32, tag="dots", bufs=2)
                nc.tensor.transpose(tp[:], xt_in[:, c * P:(c + 1) * P], ident[:])
                nc.vector.tensor_copy(xT[c][:, mt * P:(mt + 1) * P], tp[:])

        # ---- Phase 2: projections ----
        # v = x @ Wv'  (natural layout, per m-tile)
        for mt in range(NT):
            pv = psp.tile([P, DIM], F32, tag="dots", bufs=2)
            for c in range(KC):
                nc.tensor.matmul(pv[:], (xT[c][:, mt * P:(mt + 1) * P]),
                                 (w_sb["v"][c][:]),
                                 start=(c == 0), stop=(c == KC - 1))
            nc.vector.tensor_copy(v_sb[mt][:], pv[:])
        # qT = Wq'.T @ xT ; kT = Wk'.T @ xT   (per inner-chunk p, n-chunk)
        for wname, dst in (("q", qT), ("k", kT)):
            for p in range(KC):
                for nck in range(KC):
                    pq = psp.tile([P, DIM], F32, tag="dots", bufs=2)
                    for c in range(KC):
                        nc.tensor.matmul(
                            pq[:],
                            (w_sb[wname][c][:, p * P:(p + 1) * P]),
                            (xT[c][:, nck * DIM:(nck + 1) * DIM]),
                            start=(c == 0), stop=(c == KC - 1))
                    nc.vector.tensor_copy(dst[p][:, nck * DIM:(nck + 1) * DIM],
                                          pq[:])

        # outT shares slots with xT (xT dead after phase 2)
        outT = [persist.tile([P, N], F32R, tag="xot", bufs=4, name=f"outT{i}") for i in range(KC)]

        # ---- Phase 3: attention ----
        for h in range(HEADS):
            hp, hr = h // 2, (h % 2) * 64
            vh = v_sb  # head h columns: [:, h*64:(h+1)*64]
            for half in range(2):
                off = half * 1024
                po = [psp.tile([P, DIM], F32, tag="po", bufs=4, name=f"po_h{h}_f{half}_{j}") for j in range(2)]
                for mt in range(NT):
                    dts = psp.tile([P, 1024], F32, tag="dots", bufs=2)
                    for j in range(2):
                        nc.tensor.matmul(
                            dts[:, j * DIM:(j + 1) * DIM],
                            (kT[hp][hr:hr + 64, mt * P:(mt + 1) * P]),
                            (qT[hp][hr:hr + 64, off + j * DIM:off + (j + 1) * DIM]),
                            start=True, stop=True)
                    at = attnp.tile([P, 1024], BF16, tag="at")
                    nc.scalar.activation(at[:], dts[:], EXP)
                    for j in range(2):
                        nc.tensor.matmul(
                            po[j][0:64, :],
                            (vh[mt][:, h * 64:(h + 1) * 64]),
                            (at[:, j * DIM:(j + 1) * DIM]),
                            start=(mt == 0), stop=(mt == NT - 1),
                            tile_position=(0, 0))
                        nc.tensor.matmul(
                            po[j][64:128, :],
                            (ones64[:]),
                            (at[:, j * DIM:(j + 1) * DIM]),
                            start=(mt == 0), stop=(mt == NT - 1),
                            tile_position=(0, 64))
                for j in range(2):
                    rc = yout.tile([64, DIM], F32, tag="sums", bufs=2,
                                   name=f"rc_h{h}_f{half}_{j}")
                    nc.vector.reciprocal(rc[:], po[j][64:128, :])
                    nc.vector.tensor_tensor(
                        out=outT[hp][hr:hr + 64, off + j * DIM:off + (j + 1) * DIM],
                        in0=po[j][0:64, :], in1=rc[:], op=MULT)

        # ---- Phase 4: y = outT.T @ Wo + bo ----
        for nt in range(NT):
            py = psp.tile([P, DIM], F32, tag="dots", bufs=2)
            for c in range(KC):
                nc.tensor.matmul(py[:], (outT[c][:, nt * P:(nt + 1) * P]),
                                 (w_sb["o"][c][:]),
                                 start=(c == 0), stop=(c == KC - 1))
            y_sb = yout.tile([P, DIM], F32)
            nc.vector.tensor_tensor(out=y_sb[:], in0=py[:], in1=bo_bc[:], op=ADD)
            nc.sync.dma_start(out_d[nt * P:(nt + 1) * P, :], y_sb[:])

    nc.compile()
    return nc


def _get_nc():
    if "nc" not in _CACHE:
        _CACHE["nc"] = _build()
    return _CACHE["nc"]


def kernel(x, Wq, Wk, Wv, sel, Wo, bo):
    from concourse.bass_utils import run_bass_kernel_spmd

    x = np.asarray(x, dtype=np.float32)
    sel = np.asarray(sel, dtype=np.float32)
    scale = float(DHEAD) ** -0.5
    wq_f = np.ascontiguousarray(np.asarray(Wq, np.float32) * sel[None, :] * scale)
    wk_f = np.ascontiguousarray(np.asarray(Wk, np.float32) * sel[None, :])
    wv_f = np.ascontiguousarray(np.asarray(Wv, np.float32) * sel[None, :])
    wo_f = np.ascontiguousarray(np.asarray(Wo, np.float32))
    bo_f = np.ascontiguousarray(np.asarray(bo, np.float32).reshape(1, DIM))

    nc = _get_nc()
    in_maps = [
        {"x": np.ascontiguousarray(x[b]), "Wq": wq_f, "Wk": wk_f,
         "Wv": wv_f, "Wo": wo_f, "bo": bo_f}
        for b in range(B)
    ]
    res = run_bass_kernel_spmd(nc, in_maps, core_ids=list(range(NCORES)))
    return np.stack([res.results[b]["out"] for b in range(B)], axis=0)
